# revision 31
# baseline (speedup 1.0000x reference)
"""Trainium2 Bass kernel for a dense transformer block (pre-LN, causal MHA + FFN).

Shapes (hardcoded): x [1024, 64, 384] fp32, 6 heads x 64, FFN hidden 1536.
Strategy: data-parallel over batch across 8 NeuronCores (128 seqs/core), no
collectives. Per core, one fused loop over segments of 8 token tiles
(16 sequences): LN1 -> QKV -> causal attention -> proj+residual -> LN2 ->
FFN+residual.

GEMM dtypes: q/k and W1 run fp8 e4m3 DoubleRow with the C=384 contraction
zero-padded to 512 so it splits into two clean 256-row pairs (xnF/xn2F keep a
permanently-zero 4th chunk on persistent buffers; mixing DR with normal
matmuls costs ~100ns/switch, and 3-chunk contractions can't pair). v runs
fp8 without DR (same speed as bf16, saves a separate bf16 xnF copy); Wo runs
bf16 (its stationary operand would be an activation pair whose 256-col
DoubleRow weight load exceeds the matmul saving). W2 keeps the baseline's
fp8 DR pairs. All fp8 weights are scaled x64 (0.02-scale weights would sit
in e4m3 subnormals); every scale is a power of two folded into existing
drain ops (exp absorbs the q,k scales via scale=SCALE/4096; the W2 residual
add rescales by 1/4096 in the same DVE op), so no extra instructions.

Attention: per (head, tile) ONE full-block S^T matmul (kF block [64d, 128s]
vs qF [64d, 128t], both seqs of the tile merged; off-diagonal cross-seq
garbage is zeroed by the same multiply that applies the causal mask, now a
single [128,768] GPSIMD op per tile) and ONE AV matmul per head (em block
[128s,128t] stationary -> FWL weight load, moving vaug [128,65]). This
halves attention's PE instruction count vs per-seq 64-wide blocks. Head
parity still owns a PSUM tile so adjacent S matmuls alternate PE row groups
(a bank may take different row groups only in different partition ranges --
HW-probed). The softmax denominator comes free as a ones-column in the v
operand; 1/l is computed as exp(-ln l) on ACT (both functions live in the
pinned act table; DVE reciprocal is 5x slower). Softmax skips the
max-subtraction (logits are O(0.2) by construction).

The residual stream (x, x2) is bf16: halves the x DMA and doubles bn_stats
throughput; rel-err cost is ~1e-3. LN affine params are folded into the
weights host-side. LN scalar chains are batched [P,8] per segment and
hoisted into the previous segment's pipeline; norm+transpose tiles
interleave with q/k g-blocks so the PE never waits on a long serial
normalize phase. x is DMA'd once per token.
"""

import os
import sys

import numpy as np

for _p in ("/opt/trn_rl_repo", os.path.expanduser("~/.axon_site/_ro/trn_rl_repo")):
    if os.path.isdir(_p) and _p not in sys.path:
        sys.path.insert(0, _p)

import ml_dtypes  # noqa: E402
import concourse.hw_specs as _hw_specs  # noqa: E402
import concourse.bacc as bacc  # noqa: E402
import concourse.tile as tile  # noqa: E402
from concourse import mybir  # noqa: E402
from concourse.bass_utils import run_bass_kernel_spmd  # noqa: E402

# Pin every activation function this kernel uses (Exp/Ln/Identity/Copy/Relu)
# to the one act table that contains them all (natural_log_exp_and_others).
# The default per-instruction table choice ping-pongs between tables, costing
# a ~1.3us table reload per switch on the ACT engine.
_ACT_PIN = {mybir.ActivationFunctionType.Exp, mybir.ActivationFunctionType.Ln,
            mybir.ActivationFunctionType.Identity,
            mybir.ActivationFunctionType.Copy,
            mybir.ActivationFunctionType.Relu}
_orig_get_tables = _hw_specs.get_activation_tables


def _pinned_tables(arch):
    out = {}
    for name, fns in _orig_get_tables(arch).items():
        out[name] = fns if name == "natural_log_exp_and_others" \
            else fns - _ACT_PIN
    return out


_hw_specs.get_activation_tables = _pinned_tables
bacc.get_activation_tables = _pinned_tables

BF16 = mybir.dt.bfloat16
F32 = mybir.dt.float32
F8 = mybir.dt.float8e4
ACTF = mybir.ActivationFunctionType
ALU = mybir.AluOpType
DR = mybir.MatmulPerfMode.DoubleRow

N_CORES = 8
B_FULL, T, C, H, D = 1024, 64, 384, 6, 64
J = 4 * C                       # 1536
B_LOC = B_FULL // N_CORES       # 128 sequences per core
NTOK = B_LOC * T                # 8192 tokens per core
P = 128
NT = NTOK // P                  # 64 token tiles (each tile = one pair of seqs)
SEG = 8                         # token tiles per fused segment
KC = C // P                     # 3 real contraction chunks over C
KP = KC + 1                     # 4 chunks with zero padding (DR pairs)
JC = J // P                     # 12 chunks over FFN hidden
EPS = 1e-5
SCALE = D ** -0.5
WS = 64.0                       # fp8 weight scale
WS2 = WS * WS                   # 4096
E4MAX = 240.0                   # TRN e4m3 max normal

_CACHE = {}
last_exec_time_ns = None
last_result = None


def _build(has_bv, has_bo, has_b2, nt=NT, loop_n=1):
    assert nt % SEG == 0
    nc = bacc.Bacc("TRN2", target_bir_lowering=False, debug=False)
    ntok = nt * P
    nseg = nt // SEG
    SW = SEG * P                # tokens per segment (1024)

    x_d = nc.dram_tensor("x", [ntok, C], BF16, kind="ExternalInput").ap()
    wq_d = nc.dram_tensor("wq", [P, KP * C], F8, kind="ExternalInput").ap()
    wk_d = nc.dram_tensor("wk", [P, KP * C], F8, kind="ExternalInput").ap()
    wv_d = nc.dram_tensor("wv", [P, KP * C], F8, kind="ExternalInput").ap()
    wo_d = nc.dram_tensor("wo", [P, KP * C], F8, kind="ExternalInput").ap()
    w1_d = nc.dram_tensor("w1", [P, KP * J], F8, kind="ExternalInput").ap()
    w2_d = nc.dram_tensor("w2", [P, JC * C], F8, kind="ExternalInput").ap()
    bq_d = nc.dram_tensor("bq", [P, KC], F32, kind="ExternalInput").ap()
    bk_d = nc.dram_tensor("bk", [P, KC], F32, kind="ExternalInput").ap()
    bh_d = nc.dram_tensor("bh", [P, JC], F32, kind="ExternalInput").ap()
    bv_d = nc.dram_tensor("bv", [1, C], BF16, kind="ExternalInput").ap()
    bo_d = nc.dram_tensor("bo_r", [1, C], BF16, kind="ExternalInput").ap()
    b2_d = nc.dram_tensor("b2_r", [1, C], BF16, kind="ExternalInput").ap()
    id_d = nc.dram_tensor("ident", [P, P], BF16, kind="ExternalInput").ap()
    mk_d = nc.dram_tensor("maskt", [P, H * 2 * T], BF16,
                          kind="ExternalInput").ap()
    out_d = nc.dram_tensor("out", [ntok, C], F32, kind="ExternalOutput").ap()

    with tile.TileContext(nc) as tc:
        with tc.tile_pool(name="singles", bufs=1) as sg, \
             tc.tile_pool(name="seg", bufs=2) as sgp, \
             tc.tile_pool(name="work", bufs=5) as wk, \
             tc.tile_pool(name="psum", bufs=1, space="PSUM") as ps:

            # ---- resident weights / constants ----
            wq_fl = sg.tile([P, KP * C], F8, name="wq")
            wk_fl = sg.tile([P, KP * C], F8, name="wk")
            w1_fl = sg.tile([P, KP * J], F8, name="w1")
            wv_fl = sg.tile([P, KP * C], F8, name="wv")
            wo_fl = sg.tile([P, KP * C], F8, name="wo")
            w2_sb = sg.tile([P, JC * C], F8, name="w2")
            nc.gpsimd.dma_start(out=wq_fl, in_=wq_d)
            nc.gpsimd.dma_start(out=wk_fl, in_=wk_d)
            nc.gpsimd.dma_start(out=w1_fl, in_=w1_d)
            nc.gpsimd.dma_start(out=wv_fl, in_=wv_d)
            nc.gpsimd.dma_start(out=wo_fl, in_=wo_d)
            nc.gpsimd.dma_start(out=w2_sb, in_=w2_d)
            wq_sb = wq_fl.rearrange("p (k c) -> p k c", k=KP)
            wk_sb = wk_fl.rearrange("p (k c) -> p k c", k=KP)
            w1_sb = w1_fl.rearrange("p (k c) -> p k c", k=KP)
            wv_sb = wv_fl.rearrange("p (k c) -> p k c", k=KP)
            wo_sb = wo_fl.rearrange("p (k c) -> p k c", k=KP)
            bq_sb = sg.tile([P, KC], F32)
            bk_sb = sg.tile([P, KC], F32)
            bh_sb = sg.tile([P, JC], F32)
            ident = sg.tile([P, P], BF16)
            maskt = sg.tile([P, H * 2 * T], BF16)
            nc.sync.dma_start(out=ident, in_=id_d)
            nc.scalar.dma_start(out=bq_sb, in_=bq_d)
            nc.scalar.dma_start(out=bk_sb, in_=bk_d)
            nc.scalar.dma_start(out=bh_sb, in_=bh_d)
            nc.scalar.dma_start(out=maskt, in_=mk_d)
            eps_sb = sg.tile([P, 1], F32)
            nc.vector.memset(eps_sb, EPS)
            ones1 = sg.tile([1, P], BF16)
            nc.vector.memset(ones1, 1.0)
            # vaug ones-column written once; v copies never touch it
            vaug_bufs = [sg.tile([P, SEG, H, D + 1], BF16, name=f"vaugb{i}")
                         for i in range(2)]
            for _vb in vaug_bufs:
                nc.vector.memset(_vb[:, :, :, D:D + 1], 1.0)
            # feature-major LN outputs, fp8, 4th chunk permanently zero
            xnF_bufs = [sg.tile([P, KP, SW], F8, name=f"xnFb{i}")
                        for i in range(2)]
            xn2F_bufs = [sg.tile([P, KP, SW], F8, name=f"xn2Fb{i}")
                         for i in range(2)]
            aoF_bufs = [sg.tile([P, KP, P], F8, name=f"aoFb{i}")
                        for i in range(3)]
            for _fb in xnF_bufs + xn2F_bufs + aoF_bufs:
                nc.vector.memset(_fb[:, KP - 1, :], 0.0)
            bv_sb = sg.tile([1, C], BF16)
            bo_sb = sg.tile([1, C], BF16)
            b2_sb = sg.tile([1, C], BF16)
            if has_bv:
                nc.sync.dma_start(out=bv_sb, in_=bv_d)
            if has_bo:
                nc.sync.dma_start(out=bo_sb, in_=bo_d)
            if has_b2:
                nc.sync.dma_start(out=b2_sb, in_=b2_d)

            def new_state(s):
                st_ = {"i0": s * SEG}
                st_["vaug"] = vaug_bufs[s % 2]
                st_["xnF"] = xnF_bufs[s % 2]
                st_["xn2F"] = xn2F_bufs[s % 2]
                st_["qF"] = [sgp.tile([P, SW], BF16, tag=f"qF{m}",
                                      name=f"qF{m}") for m in range(KC)]
                st_["kF"] = [sgp.tile([P, SW], BF16, tag=f"kF{m}",
                                      name=f"kF{m}") for m in range(KC)]
                st_["attn"] = sgp.tile([P, SEG * C], BF16, tag="attn",
                                       name="attn")
                st_["x2"] = sgp.tile([P, SEG, C], BF16, tag="x2", name="x2")
                st_["mvA"] = sgp.tile([P, 2 * SEG], F32, tag="mvA", name="mvA")
                st_["mvD"] = sgp.tile([P, 2 * SEG], F32, tag="mvD", name="mvD")
                st_["xa"] = sgp.tile([P, SEG, C], BF16, tag="xa", name="xa")
                return st_

            # ---------- LN helpers ----------
            def emit_stats(st_, t, which):
                """DMA (LN1 only) + bn stats for tile t -> mv[:, 2t:2t+2]."""
                if which == "A":
                    xt = st_["xa"][:, t, :]
                    nc.sync.dma_start(
                        out=xt,
                        in_=x_d[(st_["i0"] + t) * P:(st_["i0"] + t + 1) * P, :])
                else:
                    xt = st_["x2"][:, t, :]
                stats = wk.tile([P, 6], F32, tag="lnstats")
                nc.vector.bn_stats(out=stats, in_=xt)
                nc.vector.bn_aggr(out=st_["mv" + which][:, 2 * t:2 * t + 2],
                                  in_=stats)

            def emit_ln_batch(st_, which, lo=0, hi=SEG):
                """Batched ln/exp/neg-mu*rstd over tiles [lo, hi) of a
                segment. rstd = exp(-0.5*ln(var+eps)) keeps every ACT op in
                the one natural_log_exp_and_others table (no table reloads).
                Split halves let the next phase's norms start before the
                last tiles' stats land."""
                n = hi - lo
                mv = st_["mv" + which].rearrange("p (t two) -> p t two",
                                                 two=2)[:, lo:hi, :]
                lnv = wk.tile([P, n], F32, tag="lnv")
                nc.scalar.activation(out=lnv, in_=mv[:, :, 1], func=ACTF.Ln,
                                     bias=eps_sb, scale=1.0)
                rstd = wk.tile([P, n], F32, tag=f"rstd{which}{lo}", bufs=2,
                               name="rstd")
                nc.scalar.activation(out=rstd, in_=lnv, func=ACTF.Exp,
                                     bias=0.0, scale=-0.5)
                mr = wk.tile([P, n], F32, tag="mr")
                nc.gpsimd.tensor_tensor(out=mr, in0=mv[:, :, 0], in1=rstd,
                                        op=ALU.mult)
                nmur = wk.tile([P, n], F32, tag=f"nmur{which}{lo}", bufs=2,
                               name="nmur")
                nc.gpsimd.tensor_scalar(out=nmur, in0=mr, scalar1=-1.0,
                                        scalar2=None, op0=ALU.mult)
                st_[f"ln{which}{lo}"] = (rstd, nmur, lo)
                if hi == SEG:
                    st_["lnb" + which] = True

            def emit_norm_tp(st_, t, which, dstF, eng, xeng=0):
                """xn0 = (x*rstd + nmur) -> bf16; one XBAR DMA transpose into
                a bf16 staging tile; cast-copy into the fp8 feature-major
                dstF (chunk 3 stays 0)."""
                src = st_["xa"][:, t, :] if which == "A" else st_["x2"][:, t, :]
                for lo in (0, 4):
                    if f"ln{which}{lo}" in st_ and lo <= t:
                        rstd, nmur, base = st_[f"ln{which}{lo}"]
                t_rel = t - base
                rstd = rstd[:, t_rel:t_rel + 1]
                nmur = nmur[:, t_rel:t_rel + 1]
                xn0 = wk.tile([P, C], BF16, tag="xn0", bufs=4, name="xn0")
                if xeng == 0:
                    nc.gpsimd.tensor_scalar(out=xn0, in0=src,
                                            scalar1=rstd, scalar2=nmur,
                                            op0=ALU.mult, op1=ALU.add)
                else:
                    nc.scalar.activation(out=xn0, in_=src, func=ACTF.Identity,
                                         bias=nmur, scale=rstd)
                tp = ps.tile([P, C], BF16, tag="big", bufs=2, name="tp")
                for k in range(KC):
                    nc.tensor.transpose(tp[:, k * P:(k + 1) * P],
                                        xn0[:, k * P:(k + 1) * P], ident)
                tpv = tp.rearrange("p (k c) -> p k c", c=P)
                dst = dstF[:, 0:KC, t * P:(t + 1) * P]
                if eng == 0:
                    nc.scalar.copy(out=dst, in_=tpv)
                else:
                    nc.vector.tensor_copy(out=dst, in_=tpv)

            # ---------- segment head: finish LNs, QKV projections ----------
            # Norm tiles interleave with q/k g-blocks so the PE starts on
            # QKV matmuls after only 4 tiles' norms instead of 16.
            def emit_seg_head(st_, prv_):
                if "lnbA" not in st_:
                    emit_ln_batch(st_, "A")
                if prv_ is not None and "lnbD" not in prv_:
                    emit_ln_batch(prv_, "D")
                for t in range(st_.pop("a_pre", 0), 4):
                    emit_norm_tp(st_, t, "A", st_["xnF"], t % 2)
                if prv_ is not None:
                    for t in range(prv_.pop("d_pre", 0), 4):
                        emit_norm_tp(prv_, t, "D", prv_["xn2F"], (t + 1) % 2)
                emit_qk(st_, 0)
                for t in range(4, SEG):
                    emit_norm_tp(st_, t, "A", st_["xnF"], t % 2)
                if prv_ is not None:
                    for t in range(4, SEG):
                        emit_norm_tp(prv_, t, "D", prv_["xn2F"], (t + 1) % 2)
                emit_qk(st_, 1)
                for t in range(SEG):
                    emit_v(st_, t)

            def emit_qk(st_, g):
                # fp8 DoubleRow pairs over the padded 512-row contraction;
                # qF/kF hold the raw 64x-scaled psum (exp absorbs the scale).
                xnF, qF, kF = st_["xnF"], st_["qF"], st_["kF"]
                for m in range(KC):
                    for wsb, dstF, bias, eng in (
                            (wq_sb, qF, bq_sb, 0), (wk_sb, kF, bk_sb, 1)):
                        pqk = ps.tile([P, 512], F32, tag="st", bufs=2)
                        for pr in range(KP // 2):
                            nc.tensor.matmul(
                                pqk,
                                wsb[:, 2 * pr:2 * pr + 2, m * P:(m + 1) * P],
                                xnF[:, 2 * pr:2 * pr + 2,
                                    g * 512:(g + 1) * 512],
                                start=(pr == 0), stop=(pr == KP // 2 - 1),
                                perf_mode=DR)
                        dst = dstF[m][:, g * 512:(g + 1) * 512]
                        if (m + 3 * g + eng) % 3 != 0:
                            nc.scalar.activation(
                                out=dst, in_=pqk, func=ACTF.Identity,
                                bias=bias[:, m:m + 1], scale=1.0)
                        else:
                            nc.vector.tensor_scalar(
                                out=dst, in0=pqk, scalar1=bias[:, m:m + 1],
                                scalar2=None, op0=ALU.add)

            def emit_v(st_, t):
                # v projection: fp8 DR pairs, xnF stationary -> token-major
                # 64x-scaled v, ones col for the denominator
                vaug = st_["vaug"]
                xnF = st_["xnF"]
                pvf = ps.tile([P, 512], F32, tag="vf", bufs=2)
                pv = pvf[:, 0:C]
                for pr in range(KP // 2):
                    nc.tensor.matmul(
                        pv, xnF[:, 2 * pr:2 * pr + 2, t * P:(t + 1) * P],
                        wv_sb[:, 2 * pr:2 * pr + 2, :], start=(pr == 0),
                        stop=(pr == KP // 2 - 1 and not has_bv),
                        perf_mode=DR)
                if has_bv:
                    nc.tensor.matmul(pv, ones1, bv_sb, start=False, stop=True)
                if t % 2 == 0:
                    nc.scalar.copy(out=vaug[:, t, :, 0:D],
                                   in_=pv.rearrange("p (h d) -> p h d", h=H))
                else:
                    nc.vector.tensor_copy(
                        out=vaug[:, t, :, 0:D],
                        in_=pv.rearrange("p (h d) -> p h d", h=H))

            # ---------- attention ----------
            def emit_S1a(st_, t):
                qF, kF = st_["qF"], st_["kF"]
                # One full-block S^T matmul per head: kF block [64d, 128s]
                # (both seqs) vs qF [64d, 128t]. Head-parity hp owns a PSUM
                # tile; adjacent matmuls alternate PE row groups so pairs run
                # concurrently. Cross-seq garbage is zeroed by the mask.
                sth = [ps.tile([P, KC * P], F32, tag="st", bufs=2, name="sth")
                       for _ in range(2)]
                for ch in range(KC):
                    for hp in range(2):
                        c0 = t * P
                        nc.tensor.matmul(
                            sth[hp][:, ch * P:(ch + 1) * P],
                            kF[ch][hp * T:(hp + 1) * T, c0:c0 + P],
                            qF[ch][hp * T:(hp + 1) * T, c0:c0 + P],
                            start=True, stop=True)
                em = wk.tile([P, 2 * KC * P], BF16, tag="em", bufs=5,
                             name="em")
                for hp in range(2):
                    nc.scalar.activation(
                        out=em[:, hp * KC * P:(hp + 1) * KC * P],
                        in_=sth[hp], func=ACTF.Exp, bias=0.0,
                        scale=SCALE / WS2)
                nc.gpsimd.tensor_tensor(out=em, in0=em, in1=maskt,
                                        op=ALU.mult)
                st_[f"em{t}"] = em

            def emit_S1b(st_, t):
                vaug, attn = st_["vaug"], st_["attn"]
                em = st_.pop(f"em{t}")
                avf = ps.tile([P, 512], F32, tag="avpr", bufs=2)
                av = avf[:, 0:H * (D + 1)].rearrange("p (h e) -> p h e",
                                                     e=D + 1)
                for ch in range(KC):
                    for hp in range(2):
                        h = 2 * ch + hp
                        nc.tensor.matmul(
                            av[:, h, :],
                            em[:, (hp * KC + ch) * P:(hp * KC + ch + 1) * P],
                            vaug[:, t, h, :],
                            start=True, stop=True)
                # 1/l as exp(-ln l): both funcs sit in the pinned ACT table
                # and ACT reads PSUM faster than DVE's iterative reciprocal.
                lnl = wk.tile([P, H], F32, tag="lnl")
                nc.scalar.activation(
                    out=lnl, in_=av[:, :, D:D + 1].rearrange("p h 1 -> p h"),
                    func=ACTF.Ln, bias=0.0, scale=1.0)
                invl = wk.tile([P, H], F32, tag="invl")
                nc.scalar.activation(out=invl, in_=lnl, func=ACTF.Exp,
                                     bias=0.0, scale=-1.0)
                nc.vector.tensor_mul(
                    out=attn[:, t * C:(t + 1) * C].rearrange(
                        "p (h d) -> p h d", h=H),
                    in0=av[:, :, 0:D],
                    in1=invl.unsqueeze(2).broadcast_to([P, H, D]))

            # ---------- proj + residual + LN2 stats ----------
            def emit_D_tile(st_, t):
                attn, x2 = st_["attn"], st_["x2"]
                tp = ps.tile([P, C], BF16, tag="big", bufs=2)
                for k in range(KC):
                    nc.tensor.transpose(
                        tp[:, k * P:(k + 1) * P],
                        attn[:, t * C + k * P: t * C + (k + 1) * P], ident)
                tpv = tp.rearrange("p (k c) -> p k c", c=P)
                aoF = aoF_bufs[(st_["i0"] + t) % 3]
                if t % 2 == 0:
                    nc.scalar.copy(out=aoF[:, 0:2, :], in_=tpv[:, 0:2, :])
                    nc.vector.tensor_copy(out=aoF[:, 2, :], in_=tpv[:, 2, :])
                else:
                    nc.vector.tensor_copy(out=aoF[:, 0:2, :],
                                          in_=tpv[:, 0:2, :])
                    nc.scalar.copy(out=aoF[:, 2, :], in_=tpv[:, 2, :])
                pprf = ps.tile([P, 512], F32, tag="avpr", bufs=2)
                ppr = pprf[:, 0:C]
                for pr in range(KP // 2):
                    nc.tensor.matmul(ppr, aoF[:, 2 * pr:2 * pr + 2, :],
                                     wo_sb[:, 2 * pr:2 * pr + 2, :],
                                     start=(pr == 0),
                                     stop=(pr == KP // 2 - 1 and not has_bo),
                                     perf_mode=DR)
                if has_bo:
                    nc.tensor.matmul(ppr, ones1, bo_sb, start=False, stop=True)
                # attn and wo both carry a 64x scale: x2 = xa + ppr/4096
                nc.vector.scalar_tensor_tensor(
                    out=x2[:, t, :], in0=ppr, scalar=1.0 / WS2,
                    in1=st_["xa"][:, t, :], op0=ALU.mult, op1=ALU.add)
                emit_stats(st_, t, "D")

            # ---------- FFN ----------
            def emit_EF_chunk(st_, ph):
                g = ph // 4
                sub = ph % 4
                if sub == 0:
                    emit_E(st_, g, 0, JC // 2)
                elif sub == 1:
                    emit_E(st_, g, JC // 2, JC)
                elif sub == 2:
                    emit_F(st_, g, 0, 2)
                else:
                    emit_F(st_, g, 2, 4)

            def emit_E(st_, g, j0, j1):
                # W1 fp8 DoubleRow pairs; psum is 64x pre-act, relu keeps the
                # scale (bh pre-scaled x64) so hFg = 64h feeds W2 directly.
                xn2F = st_["xn2F"]
                hFg = st_.setdefault(
                    f"hF{g}", sgp.tile([P, JC, 512], F8, tag=f"hF{g}",
                                       name=f"hF{g}"))
                for j in range(j0, j1):
                    phf = ps.tile([P, 512], F32, tag="big", bufs=2)
                    for pr in range(KP // 2):
                        nc.tensor.matmul(
                            phf,
                            w1_sb[:, 2 * pr:2 * pr + 2, j * P:(j + 1) * P],
                            xn2F[:, 2 * pr:2 * pr + 2,
                                 g * 512:(g + 1) * 512],
                            start=(pr == 0), stop=(pr == KP // 2 - 1),
                            perf_mode=DR)
                    if j % 3 != 0:
                        nc.scalar.activation(out=hFg[:, j, :], in_=phf,
                                             func=ACTF.Relu,
                                             bias=bh_sb[:, j:j + 1], scale=1.0)
                    else:
                        nc.vector.tensor_scalar(out=hFg[:, j, :], in0=phf,
                                                scalar1=bh_sb[:, j:j + 1],
                                                scalar2=0.0, op0=ALU.add,
                                                op1=ALU.max)

            def emit_F(st_, g, tg0, tg1):
                i0, x2 = st_["i0"], st_["x2"]
                hFg = st_[f"hF{g}"]
                w2v = w2_sb.rearrange("p (j c) -> p j c", j=JC)
                for tg in range(tg0, tg1):
                    t = g * 4 + tg
                    pff = ps.tile([P, 512], F32, tag="vf", bufs=2)
                    pf = pff[:, 0:C]
                    # 6 clean DoubleRow pairs; psum = 4096*(h @ W2)
                    for pr in range(JC // 2):
                        nc.tensor.matmul(
                            pf,
                            hFg[:, 2 * pr:2 * pr + 2, tg * P:(tg + 1) * P],
                            w2v[:, 2 * pr:2 * pr + 2, :],
                            start=(pr == 0),
                            stop=(pr == JC // 2 - 1 and not has_b2),
                            perf_mode=DR)
                    if has_b2:
                        nc.tensor.matmul(pf, ones1, b2_sb, start=False,
                                         stop=True)
                    # ot = pf/4096 + x2, exact
                    ot = wk.tile([P, C], F32, tag="ot")
                    nc.vector.scalar_tensor_tensor(
                        out=ot, in0=pf, scalar=1.0 / WS2, in1=x2[:, t, :],
                        op0=ALU.mult, op1=ALU.add)
                    nc.sync.dma_start(
                        out=out_d[(i0 + t) * P:(i0 + t + 1) * P, :], in_=ot)

            # ====== software-pipelined emission over segments ======
            def _emit_all():
                cur = new_state(0)
                for t in range(SEG):
                    emit_stats(cur, t, "A")
                prv = None
                for s in range(nseg):
                    emit_seg_head(cur, prv)
                    nxt = new_state(s + 1) if s + 1 < nseg else None
                    for t in range(SEG + 5):
                        if t < SEG:
                            emit_S1a(cur, t)
                        if 2 <= t <= SEG + 1:
                            emit_S1b(cur, t - 2)
                        if 3 <= t <= SEG + 2:
                            emit_D_tile(cur, t - 3)
                        if nxt is not None and t < SEG:
                            emit_stats(nxt, t, "A")
                        if nxt is not None and t == 4:
                            emit_ln_batch(nxt, "A", 0, 4)
                        if nxt is not None and t == SEG:
                            emit_ln_batch(nxt, "A", 4, SEG)
                        if t == 7:
                            emit_ln_batch(cur, "D", 0, 4)
                        if t == SEG + 3:
                            emit_ln_batch(cur, "D", 4, SEG)
                        # pre-warm the next segment's first norm/transpose
                        # tiles into the pipeline tail (big-tag psum is free
                        # here) so the PE has work across the segment seam
                        if nxt is not None and SEG + 1 <= t <= SEG + 4:
                            # casts on ACT: at the seam DVE is backlogged
                            # (bn_stats + drains) while ACT idles, and these
                            # casts gate the next segment's qk(0)
                            emit_norm_tp(nxt, t - SEG - 1, "A", nxt["xnF"], 0)
                            nxt["a_pre"] = t - SEG
                        if prv is not None and t < 8:
                            emit_EF_chunk(prv, t)
                    prv, cur = cur, nxt
                # tail: last segment's LN2 + FFN
                if "lnbD" not in prv:
                    emit_ln_batch(prv, "D")
                for t in range(prv.pop("d_pre", 0), SEG):
                    emit_norm_tp(prv, t, "D", prv["xn2F"], (t + 1) % 2)
                for ph in range(8):
                    emit_EF_chunk(prv, ph)

            import contextlib
            loop_ctx = tc.For_i(0, loop_n) if loop_n > 1 \
                else contextlib.nullcontext()
            with loop_ctx:
                _emit_all()

    nc.compile()
    return nc


def _bf16(a):
    return np.asarray(a, np.float32).astype(ml_dtypes.bfloat16)


def _f8(a):
    return np.clip(np.asarray(a, np.float32), -E4MAX,
                   E4MAX).astype(ml_dtypes.float8_e4m3)


def _pad_pack(w, cols):
    """[C, cols] -> padded [KP*P, cols] -> [P, KP*cols] chunk-plane layout."""
    wp = np.zeros((KP * P, cols), np.float32)
    wp[:C] = w
    return wp.reshape(KP, P, cols).transpose(1, 0, 2).reshape(P, KP * cols)


def _prep(ln1_g, ln1_b, Wq, Wk, Wv, Wo, bo, ln2_g, ln2_b, W1, b1, W2, b2):
    """Host-side weight prep: fold LN affine into weights, pack aux consts."""
    ln1_g = np.asarray(ln1_g, np.float32)
    ln1_b = np.asarray(ln1_b, np.float32)
    ln2_g = np.asarray(ln2_g, np.float32)
    ln2_b = np.asarray(ln2_b, np.float32)
    wq_all = np.asarray(Wq, np.float32).transpose(1, 0, 2).reshape(C, C)
    wk_all = np.asarray(Wk, np.float32).transpose(1, 0, 2).reshape(C, C)
    wv_all = np.asarray(Wv, np.float32).transpose(1, 0, 2).reshape(C, C)
    W1 = np.asarray(W1, np.float32)
    W2 = np.asarray(W2, np.float32)
    bq = WS * (ln1_b @ wq_all)
    bk = WS * (ln1_b @ wk_all)
    bv = WS * (ln1_b @ wv_all)
    bh = WS * (np.asarray(b1, np.float32) + ln2_b @ W1)
    causal_t = np.tril(np.ones((T, T), np.float32)).T  # [s, t]: s<=t allowed
    mblk = np.zeros((2 * T, 2 * T), np.float32)
    mblk[:T, :T] = causal_t
    mblk[T:, T:] = causal_t
    w2ch = (WS * W2).reshape(JC, P, C)
    w2p = np.concatenate([w2ch[j] for j in range(JC)], axis=1)
    d = {
        "wq": _f8(_pad_pack(WS * ln1_g[:, None] * wq_all, C)),
        "wk": _f8(_pad_pack(WS * ln1_g[:, None] * wk_all, C)),
        "wv": _f8(_pad_pack(WS * ln1_g[:, None] * wv_all, C)),
        "wo": _f8(_pad_pack(WS * np.asarray(Wo, np.float32), C)),
        "w1": _f8(_pad_pack(WS * ln2_g[:, None] * W1, J)),
        "w2": _f8(w2p),
        "bq": bq.reshape(KC, P).T.copy(),
        "bk": bk.reshape(KC, P).T.copy(),
        "bh": bh.reshape(JC, P).T.copy(),
        "bv": _bf16(bv).reshape(1, C),
        "bo_r": _bf16(WS2 * np.asarray(bo, np.float32)).reshape(1, C),
        "b2_r": _bf16(WS2 * np.asarray(b2, np.float32)).reshape(1, C),
        "ident": np.eye(P, dtype=np.float32).astype(ml_dtypes.bfloat16),
        "maskt": _bf16(np.tile(mblk, (1, H))),
    }
    flags = (bool(np.any(bv != 0)), bool(np.any(np.asarray(bo) != 0)),
             bool(np.any(np.asarray(b2) != 0)))
    return d, flags


def kernel(x, ln1_g, ln1_b, Wq, Wk, Wv, Wo, bo, ln2_g, ln2_b, W1, b1, W2, b2):
    global last_exec_time_ns, last_result
    x = _bf16(np.asarray(x, np.float32))
    aux, flags = _prep(ln1_g, ln1_b, Wq, Wk, Wv, Wo, bo, ln2_g, ln2_b, W1, b1,
                       W2, b2)
    key = flags
    if key not in _CACHE:
        _CACHE[key] = _build(*flags)
    nc = _CACHE[key]
    in_maps = []
    for c in range(N_CORES):
        m = dict(aux)
        m["x"] = x[c * B_LOC:(c + 1) * B_LOC].reshape(NTOK, C)
        in_maps.append(m)
    trace = bool(os.environ.get("BASS_TRACE"))
    try:
        res = run_bass_kernel_spmd(nc, in_maps, list(range(N_CORES)),
                                   trace=trace)
    except ModuleNotFoundError:
        res = run_bass_kernel_spmd(nc, in_maps, list(range(N_CORES)))
    last_exec_time_ns = res.exec_time_ns
    last_result = res
    out = np.stack([res.results[c]["out"] for c in range(N_CORES)])
    return out.reshape(B_FULL, T, C).astype(np.float32)
